# revision 43
# baseline (speedup 1.0000x reference)
"""AgendaCopyGenerator fused kernel for one TRN2 chip (8 NeuronCores).

Computation (reference):
    logits = hidden @ W.T + b ; logits[:, 1] = -inf
    prob   = softmax(logits)
    p_copy = sigmoid(hidden @ W_copy.T + b_copy)
    out_prob  = prob * (1 - p_copy)                        # (N, 32000)
    mul_attn  = attn[:, 300:] * p_copy                     # (N, 100)
    copy_prob = einsum('tba,abv->tbv', mul_attn.reshape(T,B,A), src_map)
    return concat([out_prob, copy_prob], axis=1), p_copy

Sharding: tensor-parallel over the 32000 vocab columns (4000 per core).
Every core reads the full hidden (needed for its vocab shard) and
computes p_copy / copy_prob redundantly (tiny); the softmax denominator
is formed with one small AllReduce per 128-row tile.

Device algorithm per core (vocab shard VS=4000 columns):
  - residents in SBUF: hidden.T as bf16 (lhsT tiles), W-shard.T as bf16
  - z = hidden @ W_copy.T via 16x8 tiny matmuls -> [128p, 16m] layout
    u = exp(-z); p_copy = 1/(1+u); q = 1-p_copy = u*p_copy
  - p_copy streamed to DRAM output (row-major) and re-read as [t, b]
  - per 128-row tile m: 64 bf16 matmuls (K=1024, N-groups <=512) into
    PSUM, pad-col mask added (core 0 only, via per-core input),
    ACT exp PSUM->bf16 E with accumulated row sums, AllReduce of the
    row-sum [128] across cores, factor = q/denom, E * factor -> f32 out
  - copy_prob: per batch b one matmul (attnT_pad [100->128, t] x
    src_map[., b, .]), scaled by p_copy[t, b], written as [t, b*120+v]

kernel(**inputs) accepts the FULL unsharded inputs and returns the full
(out, p_copy) tuple exactly like the reference.
"""

import numpy as np
import ml_dtypes

import concourse.bass as bass
import concourse.mybir as mybir
import concourse.tile as tile
from concourse import bacc
from concourse.bass_utils import run_bass_kernel_spmd
from concourse.masks import make_identity

F32 = mybir.dt.float32
BF16 = mybir.dt.bfloat16

P = 128            # partitions / row-tile height
NROW = 2048        # batch*tlen rows
D = 1024           # hidden size
V = 32000          # vocab
NCORES = 8
VS = V // NCORES   # vocab shard per core (4000)
MT = NROW // P     # row tiles (16)
B = 16             # batch
T = 128            # tlen
A = 100            # agenda len
CV = 120           # copy vocab
CTX = 300          # context_len = slen - agenda
PAD_IDX = 1
HALF = 2048        # columns in first half of the vocab shard
NEG = -1.0e9
G = 4              # row tiles per softmax-denominator AllReduce

_CACHE = {}

# Row-sharded variant: 256 rows x full vocab per core, W streamed from
# HBM, softmax denominators fully local -> zero collectives.
USE_ROWS = True
VP = 32768          # vocab padded to 16 quads of QW
QW = 2048           # columns per streamed W quad / E chunk
NQ = VP // QW       # 16
RPC = NROW // NCORES  # rows per core (256)
MTC = RPC // P      # row tiles per core (2)


def _groups(width):
    """Split width into matmul N-groups of <=512 that never cross a
    512-f32 PSUM bank boundary (tile is bank aligned)."""
    out = []
    off = 0
    while off < width:
        g = min(512, width - off)
        out.append((off, g))
        off += g
    return out


def _build(kc):
    """Build + compile the SPMD graph. kc = number of 128-deep K chunks
    (8 normally; 9 when a bias row is folded in)."""
    nc = bacc.Bacc("TRN2", target_bir_lowering=False, debug=False,
                   num_devices=NCORES)

    hT_d = nc.dram_tensor("hT", [kc * P, NROW], BF16, kind="ExternalInput")
    wT_d = nc.dram_tensor("wT", [kc * P, VS], BF16, kind="ExternalInput")
    wc_d = nc.dram_tensor("wc", [P, kc], BF16, kind="ExternalInput")
    at_d = nc.dram_tensor("at", [P, B * T], BF16, kind="ExternalInput")
    sm_d = nc.dram_tensor("sm", [P, B * CV], BF16, kind="ExternalInput")
    mk_d = nc.dram_tensor("mk", [P, 1], F32, kind="ExternalInput")

    out_d = nc.dram_tensor("out0", [NROW, VS], BF16, kind="ExternalOutput")
    pc_d = nc.dram_tensor("pc", [NROW, 1], F32, kind="ExternalOutput")
    cp_d = nc.dram_tensor("cp", [P, B * CV], F32, kind="ExternalOutput")

    with tile.TileContext(nc) as tc:
        with (
            tc.tile_pool(name="res", bufs=1) as res,
            tc.tile_pool(name="small", bufs=3) as small,
            tc.tile_pool(name="epool", bufs=9) as epool,
            tc.tile_pool(name="psum", bufs=2, space="PSUM") as psum,
            tc.tile_pool(name="dram", bufs=1, space="DRAM") as dram,
        ):
            # ---- warm up the collective firmware early --------------
            # The first collective_compute in a NEFF costs ~70 us extra;
            # run a nearly dependency-free dummy AllReduce under the
            # input DMAs so later denominators take the ~13 us fast path.
            mk0_sb = res.tile([P, 1], F32, name="mk0_sb")
            nc.sync.dma_start(mk0_sb[:], mk_d[:])
            wu_in = dram.tile([P, 1], F32, name="wu_in")
            wu_out = dram.tile([P, 1], F32, name="wu_out")
            nc.gpsimd.dma_start(wu_in[:], mk0_sb[:])
            nc.gpsimd.collective_compute(
                "AllReduce", mybir.AluOpType.add,
                replica_groups=[list(range(NCORES))],
                ins=[wu_in.opt()], outs=[wu_out.opt()])

            # ---- resident loads -------------------------------------
            # Interleave the W-shard first halves with the hidden chunks
            # so row-tile 0's k-th matmul can fire as chunk k lands; the
            # second halves and the copy-path inputs stream afterwards.
            wc_sb = res.tile([P, kc], BF16, name="wc_sb")
            nc.sync.dma_start(wc_sb[:], wc_d[:])
            mk_sb = res.tile([P, 1], F32, name="mk_sb")
            nc.sync.dma_start(mk_sb[:], mk_d[:])
            ht = [res.tile([P, NROW], BF16, name=f"ht{k}") for k in range(kc)]
            wt = [res.tile([P, VS], BF16, name=f"wt{k}") for k in range(kc)]
            # hidden chunks first: the z matmuls pace on them and keep
            # the PE warm while the W shard streams in behind
            for k in range(kc):
                nc.sync.dma_start(ht[k][:], hT_d[k * P:(k + 1) * P, :])
            for k in range(kc):
                nc.sync.dma_start(wt[k][:, 0:HALF],
                                  wT_d[k * P:(k + 1) * P, 0:HALF])
            for k in range(kc):
                nc.sync.dma_start(wt[k][:, HALF:VS],
                                  wT_d[k * P:(k + 1) * P, HALF:VS])
            at_sb = res.tile([P, B * T], BF16, name="at_sb")
            nc.sync.dma_start(at_sb[:], at_d[:])
            sm_sb = res.tile([P, B * CV], BF16, name="sm_sb")
            nc.sync.dma_start(sm_sb[:], sm_d[:])

            q_pm = res.tile([P, MT], F32, name="q_pm")
            pc_tb = res.tile([P, B], F32, name="pc_tb")

            def z_block():
                # z = hidden @ W_copy.T in [p, m] layout, then
                # p_copy = 1/(1+exp(-z)), q = 1-p_copy.  p_copy leaves in
                # row-major order via a TensorE transpose (a scattered DMA
                # would cost thousands of 4-byte descriptors), then comes
                # back from DRAM in [t, b] layout — both DMAs contiguous.
                zp = psum.tile([P, HALF], F32, name="pt", tag="pt")
                for m in range(MT):
                    zcol = m * P + 124  # stays inside one PSUM bank
                    for k in range(kc):
                        nc.tensor.matmul(
                            zp[:, zcol:zcol + 1],
                            ht[k][:, m * P:(m + 1) * P],
                            wc_sb[:, k:k + 1],
                            start=(k == 0), stop=(k == kc - 1),
                        )
                zv = zp[:].rearrange("p (m c) -> p m c", c=P)[:, :, 124]
                u_pm = res.tile([P, MT], F32, name="u_pm")
                nc.scalar.activation(u_pm[:], zv,
                                     mybir.ActivationFunctionType.Exp,
                                     scale=-1.0)
                den_pm = res.tile([P, MT], F32, name="den_pm")
                nc.vector.tensor_scalar_add(den_pm[:], u_pm[:], 1.0)
                pc_pm = res.tile([P, MT], F32, name="pc_pm")
                nc.vector.reciprocal(pc_pm[:], den_pm[:])
                nc.vector.tensor_mul(q_pm[:], u_pm[:], pc_pm[:])

                ident = res.tile([P, P], F32, name="ident")
                make_identity(nc, ident[:])
                pc_ext = res.tile([P, P], F32, name="pc_ext")
                nc.vector.memset(pc_ext[:], 0.0)
                nc.vector.tensor_copy(pc_ext[:, 0:MT], pc_pm[:])
                s_ps = psum.tile([P, HALF], F32, name="pt", tag="pt")
                nc.tensor.transpose(s_ps[:, 0:P], pc_ext[:], ident[:])
                s_sb = res.tile([P, P], F32, name="s_sb")
                nc.vector.tensor_copy(s_sb[:], s_ps[:, 0:P])
                # s_sb[m, p] = p_copy[row m*128+p] -> row-major pc output
                nc.sync.dma_start(
                    pc_d.ap().rearrange("(m p) one -> m (p one)", m=MT),
                    s_sb[0:MT, :])
                nc.gpsimd.dma_start(
                    pc_tb[:],
                    pc_d.ap().rearrange("(t b) one -> t (b one)", t=P))

            def copy_path():
                # one matmul per batch; PSUM drained immediately so the
                # slot frees fast, scaled by p_copy[t, b] later
                cpt = psum.tile([P, HALF], F32, name="pt", tag="pt")
                for bb in range(B):
                    nc.tensor.matmul(
                        cpt[:, bb * P:bb * P + CV],
                        at_sb[:, bb * T:(bb + 1) * T],
                        sm_sb[:, bb * CV:(bb + 1) * CV],
                        start=True, stop=True)
                cp_raw = res.tile([P, B * P], F32, name="cp_raw")
                nc.vector.tensor_copy(cp_raw[:], cpt[:, 0:B * P])
                cp_sb = res.tile([P, B * CV], F32, name="cp_sb")
                for bb in range(B):
                    nc.vector.tensor_scalar(
                        cp_sb[:, bb * CV:(bb + 1) * CV],
                        cp_raw[:, bb * P:bb * P + CV],
                        pc_tb[:, bb:bb + 1], None, mybir.AluOpType.mult)
                nc.sync.dma_start(cp_d[:], cp_sb[:])

            # ---- phase B: vocab-shard softmax stream ----------------
            # Groups of row tiles share one denominator AllReduce; their
            # E tiles stay resident until the reduced sums return, then
            # are scaled in place and streamed out as bf16.  Groups
            # taper at the end so the final collective covers one tile
            # and the tail stays short.  Small latency-critical DMAs
            # ride the gpsimd/qAct queues, never behind the megabyte
            # streams on qSP.
            GROUPS = [4, 4, 4, 4]
            assert sum(GROUPS) == MT
            z_block()

            def scale_block(pend):
                # runs one group behind the compute stream: by now the
                # group's AllReduce has had a full group's compute time
                # to finish, so nothing below blocks an engine queue
                gi, gtiles, b_out, e_tiles = pend
                gn = len(gtiles)
                d_g = small.tile([P, G], F32, name="d_g")
                nc.gpsimd.dma_start(d_g[:, 0:gn], b_out[:])
                rec = small.tile([P, G], F32, name="rec")
                nc.vector.reciprocal(rec[:, 0:gn], d_g[:, 0:gn])
                f_g = small.tile([P, G], F32, name="f_g")
                nc.vector.tensor_mul(f_g[:, 0:gn],
                                     q_pm[:, gtiles[0]:gtiles[0] + gn],
                                     rec[:, 0:gn])
                for j, m in enumerate(gtiles):
                    e = e_tiles[m]
                    if j % 2 == 0:
                        # DVE scale, stream out on qSP
                        nc.vector.tensor_scalar(e[:], e[:], f_g[:, j:j + 1],
                                                None, mybir.AluOpType.mult)
                        nc.sync.dma_start(out_d[m * P:(m + 1) * P, :], e[:])
                    else:
                        # ACT scale, stream out on qAct — splits the
                        # serialized scale+store chain across engines
                        nc.scalar.mul(e[:], e[:], f_g[:, j:j + 1])
                        nc.scalar.dma_start(out_d[m * P:(m + 1) * P, :], e[:])

            pending = None
            g0 = 0
            for gi, gn in enumerate(GROUPS):
                gtiles = list(range(g0, g0 + gn))
                g0 += gn
                e_tiles = {}
                sg = small.tile([P, G], F32, name="sg")
                for j, m in enumerate(gtiles):
                    msl = slice(m * P, (m + 1) * P)
                    e = epool.tile([P, VS], BF16, name="e")
                    e_tiles[m] = e
                    s2 = small.tile([P, 2], F32, name="s2")
                    for h in range(2):
                        c0 = h * HALF
                        wdt = HALF if h == 0 else VS - HALF
                        pt = psum.tile([P, HALF], F32, name="pt", tag="pt")
                        for k in range(kc):
                            for off, gw in _groups(wdt):
                                nc.tensor.matmul(
                                    pt[:, off:off + gw],
                                    ht[k][:, msl],
                                    wt[k][:, c0 + off:c0 + off + gw],
                                    start=(k == 0), stop=(k == kc - 1),
                                )
                        if h == 0:
                            # PAD_IDX mask: per-core input, -1e9 on core 0
                            nc.vector.tensor_scalar(
                                pt[:, PAD_IDX:PAD_IDX + 1],
                                pt[:, PAD_IDX:PAD_IDX + 1],
                                mk_sb[:, 0:1], None, mybir.AluOpType.add)
                        nc.scalar.activation(
                            e[:, c0:c0 + wdt], pt[:, 0:wdt],
                            mybir.ActivationFunctionType.Exp,
                            accum_out=s2[:, h:h + 1])
                    nc.vector.tensor_add(sg[:, j:j + 1],
                                         s2[:, 0:1], s2[:, 1:2])
                    if m == 1:
                        # overlaps the attn/src_map DMA tail; PE is warm
                        copy_path()

                # contiguous [P, gn] bounce layout: AllReduce is
                # elementwise, so keep the SBUF-native partition-major
                # order and every DMA stays descriptor-friendly
                b_in = dram.tile([P, gn], F32, name=f"bin{gi}")
                b_out = dram.tile([P, gn], F32, name=f"bout{gi}")
                # qAct HW-DGE: fires right after the exps that made sg
                nc.scalar.dma_start(b_in[:], sg[:, 0:gn])
                nc.gpsimd.collective_compute(
                    "AllReduce", mybir.AluOpType.add,
                    replica_groups=[list(range(NCORES))],
                    ins=[b_in.opt()], outs=[b_out.opt()])
                if pending is not None:
                    scale_block(pending)
                pending = (gi, gtiles, b_out, e_tiles)
            scale_block(pending)


    nc.compile()
    return nc


def _build_rows(kc):
    """Row-sharded SPMD graph: each core owns 256 rows and the full
    (padded) vocab; W streams through SBUF in 2048-column quads; the
    softmax denominator is local, so there are no collectives at all."""
    nc = bacc.Bacc("TRN2", target_bir_lowering=False, debug=False,
                   num_devices=NCORES)

    htm_d = nc.dram_tensor("htm", [kc * P, RPC], BF16, kind="ExternalInput")
    wP_d = nc.dram_tensor("wP", [kc * P, VP], BF16, kind="ExternalInput")
    wc_d = nc.dram_tensor("wc", [P, kc], BF16, kind="ExternalInput")
    at_d = nc.dram_tensor("at", [P, B * B], BF16, kind="ExternalInput")
    sm_d = nc.dram_tensor("sm", [P, B * CV], BF16, kind="ExternalInput")

    out_d = nc.dram_tensor("out0", [RPC, V], BF16, kind="ExternalOutput")
    pc_d = nc.dram_tensor("pc", [RPC, 1], F32, kind="ExternalOutput")
    cp_d = nc.dram_tensor("cp", [B, B * CV], F32, kind="ExternalOutput")

    with tile.TileContext(nc) as tc:
        with (
            tc.tile_pool(name="res", bufs=1) as res,
            tc.tile_pool(name="small", bufs=3) as small,
            tc.tile_pool(name="epool", bufs=NQ + 6) as epool,
            tc.tile_pool(name="wqpool", bufs=2) as wqpool,
            tc.tile_pool(name="psum", bufs=2, space="PSUM") as psum,
        ):
            # ---- tiny resident loads --------------------------------
            wc_sb = res.tile([P, kc], BF16, name="wc_sb")
            nc.sync.dma_start(wc_sb[:], wc_d[:])
            htm = [res.tile([P, RPC], BF16, name=f"htm{k}")
                   for k in range(kc)]
            for k in range(kc):
                nc.sync.dma_start(htm[k][:], htm_d[k * P:(k + 1) * P, :])
            at_sb = res.tile([P, B * B], BF16, name="at_sb")
            nc.sync.dma_start(at_sb[:], at_d[:])
            sm_sb = res.tile([P, B * CV], BF16, name="sm_sb")
            nc.sync.dma_start(sm_sb[:], sm_d[:])

            q_own = res.tile([P, MTC], F32, name="q_own")
            pc_tbo = res.tile([B, B], F32, name="pc_tbo")

            def z_block():
                # z for the core's own rows -> p_copy, q, and the
                # [t_local, b] layout via TensorE transpose + roundtrip
                zp = psum.tile([P, QW], F32, name="pt", tag="pt")
                for m in range(MTC):
                    zcol = m * P + 124
                    for k in range(kc):
                        nc.tensor.matmul(
                            zp[:, zcol:zcol + 1],
                            htm[k][:, m * P:(m + 1) * P],
                            wc_sb[:, k:k + 1],
                            start=(k == 0), stop=(k == kc - 1),
                        )
                zv = zp[:].rearrange("p (m c) -> p m c", c=P)[:, 0:MTC, 124]
                u_o = res.tile([P, MTC], F32, name="u_o")
                nc.scalar.activation(u_o[:], zv,
                                     mybir.ActivationFunctionType.Exp,
                                     scale=-1.0)
                den_o = res.tile([P, MTC], F32, name="den_o")
                nc.vector.tensor_scalar_add(den_o[:], u_o[:], 1.0)
                pc_o = res.tile([P, MTC], F32, name="pc_o")
                nc.vector.reciprocal(pc_o[:], den_o[:])
                nc.vector.tensor_mul(q_own[:], u_o[:], pc_o[:])

                ident = res.tile([P, P], F32, name="ident")
                make_identity(nc, ident[:])
                pc_ext = res.tile([P, P], F32, name="pc_ext")
                nc.vector.memset(pc_ext[:], 0.0)
                nc.vector.tensor_copy(pc_ext[:, 0:MTC], pc_o[:])
                s_ps = psum.tile([P, QW], F32, name="pt", tag="pt")
                nc.tensor.transpose(s_ps[:, 0:P], pc_ext[:], ident[:])
                s_sb = res.tile([P, P], F32, name="s_sb")
                nc.vector.tensor_copy(s_sb[:], s_ps[:, 0:P])
                nc.sync.dma_start(
                    pc_d.ap().rearrange("(m p) one -> m (p one)", m=MTC),
                    s_sb[0:MTC, :])
                nc.gpsimd.dma_start(
                    pc_tbo[:],
                    pc_d.ap().rearrange("(t b) one -> t (b one)", t=B))

            def copy_path():
                # copy_prob for the core's own 16 t-values
                cpt = psum.tile([P, QW], F32, name="pt", tag="pt")
                for bb in range(B):
                    nc.tensor.matmul(
                        cpt[0:B, bb * P:bb * P + CV],
                        at_sb[:, bb * B:(bb + 1) * B],
                        sm_sb[:, bb * CV:(bb + 1) * CV],
                        start=True, stop=True)
                cp_sb = res.tile([B, B * CV], F32, name="cp_sb")
                for bb in range(B):
                    nc.vector.tensor_scalar(
                        cp_sb[:, bb * CV:(bb + 1) * CV],
                        cpt[0:B, bb * P:bb * P + CV],
                        pc_tbo[:, bb:bb + 1], None, mybir.AluOpType.mult)
                nc.sync.dma_start(cp_d[:], cp_sb[:])

            z_block()

            def scale_store(m, e_tiles, s16):
                s_m = small.tile([P, 1], F32, name="s_m")
                nc.vector.tensor_reduce(s_m[:], s16[:],
                                        mybir.AxisListType.X,
                                        mybir.AluOpType.add)
                rec = small.tile([P, 1], F32, name="rec")
                nc.vector.reciprocal(rec[:], s_m[:])
                f_m = small.tile([P, 1], F32, name="f_m")
                nc.vector.tensor_mul(f_m[:], q_own[:, m:m + 1], rec[:])
                for q in range(NQ):
                    e = e_tiles[q]
                    cw = QW if q < NQ - 1 else V - (NQ - 1) * QW
                    r0 = m * P
                    if q % 2 == 0:
                        nc.vector.tensor_scalar(e[:], e[:], f_m[:], None,
                                                mybir.AluOpType.mult)
                        nc.sync.dma_start(
                            out_d[r0:r0 + P, q * QW:q * QW + cw],
                            e[:, 0:cw])
                    else:
                        nc.scalar.mul(e[:], e[:], f_m[:])
                        nc.scalar.dma_start(
                            out_d[r0:r0 + P, q * QW:q * QW + cw],
                            e[:, 0:cw])

            # ---- main stream: 2 row tiles x 16 W quads --------------
            pend = None
            for m in range(MTC):
                msl = slice(m * P, (m + 1) * P)
                e_tiles = []
                s16 = small.tile([P, NQ], F32, name="s16")
                for q in range(NQ):
                    wq = wqpool.tile([P, kc, QW], BF16, name="wq")
                    nc.sync.dma_start(
                        wq[:],
                        wP_d[0:kc * P, q * QW:(q + 1) * QW].rearrange(
                            "(kk p) n -> p kk n", p=P))
                    pt = psum.tile([P, QW], F32, name="pt", tag="pt")
                    for k in range(kc):
                        for off, gw in _groups(QW):
                            nc.tensor.matmul(
                                pt[:, off:off + gw],
                                htm[k][:, msl],
                                wq[:, k, off:off + gw],
                                start=(k == 0), stop=(k == kc - 1),
                            )
                    if q == 0:
                        # PAD_IDX column (static: every core holds col 1)
                        nc.vector.memset(pt[:, PAD_IDX:PAD_IDX + 1], NEG)
                    if q == NQ - 1:
                        # padded vocab columns [32000:32768]
                        nc.vector.memset(pt[:, V - (NQ - 1) * QW:QW], NEG)
                    e = epool.tile([P, QW], BF16, name="e")
                    e_tiles.append(e)
                    nc.scalar.activation(
                        e[:], pt[:], mybir.ActivationFunctionType.Exp,
                        accum_out=s16[:, q:q + 1])
                    if m == 0 and q == 1:
                        copy_path()
                    if q == 2 and pend is not None:
                        # deferred: the previous row tile's scales never
                        # sit in front of this tile's exps on ACT/DVE
                        scale_store(*pend)
                        pend = None
                pend = (m, e_tiles, s16)
            scale_store(*pend)

    nc.compile()
    return nc


def _get_nc(kc):
    key = ("rows" if USE_ROWS else "vocab", kc)
    if key not in _CACHE:
        _CACHE[key] = _build_rows(kc) if USE_ROWS else _build(kc)
    return _CACHE[key]


def _prepare_in_maps(hidden, attn, src_map, W, b, W_copy, b_copy):
    hidden = np.asarray(hidden, dtype=np.float32)
    attn = np.asarray(attn, dtype=np.float32)
    src_map = np.asarray(src_map, dtype=np.float32)
    W = np.asarray(W, dtype=np.float32)
    b = np.asarray(b, dtype=np.float32)
    W_copy = np.asarray(W_copy, dtype=np.float32)
    b_copy = np.asarray(b_copy, dtype=np.float32)

    use_bias = bool(np.any(b != 0.0) or np.any(b_copy != 0.0))
    kc = D // P + (1 if use_bias else 0)

    # hidden.T (+ optional ones row for bias folding), bf16
    hT = np.zeros((kc * P, NROW), dtype=ml_dtypes.bfloat16)
    hT[:D, :] = hidden.T.astype(ml_dtypes.bfloat16)
    if use_bias:
        hT[D, :] = ml_dtypes.bfloat16(1.0)

    # W_copy.T chunks -> [P, kc]
    wc = np.zeros((P, kc), dtype=ml_dtypes.bfloat16)
    wc_full = np.zeros((kc * P,), dtype=np.float32)
    wc_full[:D] = W_copy[0, :]
    if use_bias:
        wc_full[D] = b_copy[0]
    wc[:, :] = wc_full.reshape(kc, P).T.astype(ml_dtypes.bfloat16)

    # attn[:, 300:].T in [a, (b t)] layout, padded to 128 rows
    at = np.zeros((P, B * T), dtype=ml_dtypes.bfloat16)
    # attn rows are r = t*B + b -> reshape (T, B, A) -> [a, b, t]
    a_tba = attn[:, CTX:].reshape(T, B, A)
    at[:A, :] = np.ascontiguousarray(
        a_tba.transpose(2, 1, 0)).reshape(A, B * T).astype(ml_dtypes.bfloat16)

    # src_map in [a, (b v)] layout, padded to 128 rows
    sm = np.zeros((P, B * CV), dtype=ml_dtypes.bfloat16)
    sm[:A, :] = np.ascontiguousarray(src_map).reshape(
        A, B * CV).astype(ml_dtypes.bfloat16)

    in_maps = []
    for c in range(NCORES):
        wT = np.zeros((kc * P, VS), dtype=ml_dtypes.bfloat16)
        wT[:D, :] = W[c * VS:(c + 1) * VS, :].T.astype(ml_dtypes.bfloat16)
        if use_bias:
            wT[D, :] = b[c * VS:(c + 1) * VS].astype(ml_dtypes.bfloat16)
        mk = np.zeros((P, 1), dtype=np.float32)
        if c == PAD_IDX // VS:
            mk[:, 0] = NEG if (0 <= PAD_IDX < VS) else 0.0
        in_maps.append({
            "hT": hT, "wT": wT, "wc": wc, "at": at, "sm": sm, "mk": mk,
        })
    return in_maps, kc


def _prepare_in_maps_rows(hidden, attn, src_map, W, b, W_copy, b_copy):
    hidden = np.asarray(hidden, dtype=np.float32)
    attn = np.asarray(attn, dtype=np.float32)
    src_map = np.asarray(src_map, dtype=np.float32)
    W = np.asarray(W, dtype=np.float32)
    b = np.asarray(b, dtype=np.float32)
    W_copy = np.asarray(W_copy, dtype=np.float32)
    b_copy = np.asarray(b_copy, dtype=np.float32)

    use_bias = bool(np.any(b != 0.0) or np.any(b_copy != 0.0))
    kc = D // P + (1 if use_bias else 0)

    hT = np.zeros((kc * P, NROW), dtype=ml_dtypes.bfloat16)
    hT[:D, :] = hidden.T.astype(ml_dtypes.bfloat16)
    if use_bias:
        hT[D, :] = ml_dtypes.bfloat16(1.0)

    wP = np.zeros((kc * P, VP), dtype=ml_dtypes.bfloat16)
    wP[:D, :V] = W.T.astype(ml_dtypes.bfloat16)
    if use_bias:
        wP[D, :V] = b.astype(ml_dtypes.bfloat16)

    wc = np.zeros((P, kc), dtype=ml_dtypes.bfloat16)
    wc_full = np.zeros((kc * P,), dtype=np.float32)
    wc_full[:D] = W_copy[0, :]
    if use_bias:
        wc_full[D] = b_copy[0]
    wc[:, :] = wc_full.reshape(kc, P).T.astype(ml_dtypes.bfloat16)

    sm = np.zeros((P, B * CV), dtype=ml_dtypes.bfloat16)
    sm[:A, :] = np.ascontiguousarray(src_map).reshape(
        A, B * CV).astype(ml_dtypes.bfloat16)

    a_tba = attn[:, CTX:].reshape(T, B, A)
    in_maps = []
    for c in range(NCORES):
        htm = np.ascontiguousarray(hT[:, c * RPC:(c + 1) * RPC])
        at = np.zeros((P, B * B), dtype=ml_dtypes.bfloat16)
        at[:A, :] = np.ascontiguousarray(
            a_tba[c * B:(c + 1) * B].transpose(2, 1, 0)).reshape(
                A, B * B).astype(ml_dtypes.bfloat16)
        in_maps.append({"htm": htm, "wP": wP, "wc": wc, "at": at, "sm": sm})
    return in_maps, kc


def _assemble(results):
    out = np.empty((NROW, V + CV), dtype=np.float32)
    if USE_ROWS:
        for c in range(NCORES):
            out[c * RPC:(c + 1) * RPC, :V] = \
                results[c]["out0"].astype(np.float32)
        cp = np.concatenate([results[c]["cp"] for c in range(NCORES)],
                            axis=0)
        out[:, V:] = cp.reshape(P, B, CV).reshape(NROW, CV)
        p_copy = np.concatenate([results[c]["pc"] for c in range(NCORES)],
                                axis=0)
    else:
        for c in range(NCORES):
            out[:, c * VS:(c + 1) * VS] = \
                results[c]["out0"].astype(np.float32)
        out[:, V:] = results[0]["cp"].reshape(P, B, CV).reshape(NROW, CV)
        p_copy = results[0]["pc"]
    return out, p_copy


def _run(inputs, trace=False, **kw):
    if USE_ROWS:
        in_maps, kc = _prepare_in_maps_rows(**inputs)
    else:
        in_maps, kc = _prepare_in_maps(**inputs)
    nc = _get_nc(kc)
    res = run_bass_kernel_spmd(nc, in_maps, core_ids=list(range(NCORES)),
                               trace=trace, **kw)
    return res


def kernel(hidden, attn, src_map, W, b, W_copy, b_copy):
    res = _run(dict(hidden=hidden, attn=attn, src_map=src_map, W=W, b=b,
                    W_copy=W_copy, b_copy=b_copy))
    return _assemble(res.results)


# revision 48
# speedup vs baseline: 1.6959x; 1.6959x over previous
"""AgendaCopyGenerator fused kernel for one TRN2 chip (8 NeuronCores).

Computation (reference):
    logits = hidden @ W.T + b ; logits[:, 1] = -inf
    prob   = softmax(logits)
    p_copy = sigmoid(hidden @ W_copy.T + b_copy)
    out_prob  = prob * (1 - p_copy)                        # (N, 32000)
    mul_attn  = attn[:, 300:] * p_copy                     # (N, 100)
    copy_prob = einsum('tba,abv->tbv', mul_attn.reshape(T,B,A), src_map)
    return concat([out_prob, copy_prob], axis=1), p_copy

Sharding: tensor-parallel over the 32000 vocab columns (4000 per core).
Every core reads the full hidden (needed for its vocab shard) and
computes p_copy / copy_prob redundantly (tiny); the softmax denominator
is formed with one small AllReduce per 128-row tile.

Device algorithm per core (vocab shard VS=4000 columns):
  - residents in SBUF: hidden.T as bf16 (lhsT tiles), W-shard.T as bf16
  - z = hidden @ W_copy.T via 16x8 tiny matmuls -> [128p, 16m] layout
    u = exp(-z); p_copy = 1/(1+u); q = 1-p_copy = u*p_copy
  - p_copy streamed to DRAM output (row-major) and re-read as [t, b]
  - per 128-row tile m: 64 bf16 matmuls (K=1024, N-groups <=512) into
    PSUM, pad-col mask added (core 0 only, via per-core input),
    ACT exp PSUM->bf16 E with accumulated row sums, AllReduce of the
    row-sum [128] across cores, factor = q/denom, E * factor -> f32 out
  - copy_prob: per batch b one matmul (attnT_pad [100->128, t] x
    src_map[., b, .]), scaled by p_copy[t, b], written as [t, b*120+v]

kernel(**inputs) accepts the FULL unsharded inputs and returns the full
(out, p_copy) tuple exactly like the reference.
"""

import numpy as np
import ml_dtypes

import concourse.bass as bass
import concourse.mybir as mybir
import concourse.tile as tile
from concourse import bacc
from concourse.bass_utils import run_bass_kernel_spmd
from concourse.masks import make_identity

F32 = mybir.dt.float32
BF16 = mybir.dt.bfloat16

P = 128            # partitions / row-tile height
NROW = 2048        # batch*tlen rows
D = 1024           # hidden size
V = 32000          # vocab
NCORES = 8
VS = V // NCORES   # vocab shard per core (4000)
MT = NROW // P     # row tiles (16)
B = 16             # batch
T = 128            # tlen
A = 100            # agenda len
CV = 120           # copy vocab
CTX = 300          # context_len = slen - agenda
PAD_IDX = 1
HALF = 2048        # columns in first half of the vocab shard
NEG = -1.0e9
G = 4              # row tiles per softmax-denominator AllReduce

_CACHE = {}

# Row-sharded variant: 256 rows x full vocab per core, W streamed from
# HBM, softmax denominators fully local -> zero collectives.
USE_ROWS = True
VP = 32768          # vocab padded to 16 quads of QW
QW = 2048           # columns per streamed W quad / E chunk
NQ = VP // QW       # 16
RPC = NROW // NCORES  # rows per core (256)
MTC = RPC // P      # row tiles per core (2)


def _groups(width):
    """Split width into matmul N-groups of <=512 that never cross a
    512-f32 PSUM bank boundary (tile is bank aligned)."""
    out = []
    off = 0
    while off < width:
        g = min(512, width - off)
        out.append((off, g))
        off += g
    return out


def _build(kc):
    """Build + compile the SPMD graph. kc = number of 128-deep K chunks
    (8 normally; 9 when a bias row is folded in)."""
    nc = bacc.Bacc("TRN2", target_bir_lowering=False, debug=False,
                   num_devices=NCORES)

    hT_d = nc.dram_tensor("hT", [kc * P, NROW], BF16, kind="ExternalInput")
    wT_d = nc.dram_tensor("wT", [kc * P, VS], BF16, kind="ExternalInput")
    wc_d = nc.dram_tensor("wc", [P, kc], BF16, kind="ExternalInput")
    at_d = nc.dram_tensor("at", [P, B * T], BF16, kind="ExternalInput")
    sm_d = nc.dram_tensor("sm", [P, B * CV], BF16, kind="ExternalInput")
    mk_d = nc.dram_tensor("mk", [P, 1], F32, kind="ExternalInput")

    out_d = nc.dram_tensor("out0", [NROW, VS], BF16, kind="ExternalOutput")
    pc_d = nc.dram_tensor("pc", [NROW, 1], F32, kind="ExternalOutput")
    cp_d = nc.dram_tensor("cp", [P, B * CV], F32, kind="ExternalOutput")

    with tile.TileContext(nc) as tc:
        with (
            tc.tile_pool(name="res", bufs=1) as res,
            tc.tile_pool(name="small", bufs=3) as small,
            tc.tile_pool(name="epool", bufs=9) as epool,
            tc.tile_pool(name="psum", bufs=2, space="PSUM") as psum,
            tc.tile_pool(name="dram", bufs=1, space="DRAM") as dram,
        ):
            # ---- warm up the collective firmware early --------------
            # The first collective_compute in a NEFF costs ~70 us extra;
            # run a nearly dependency-free dummy AllReduce under the
            # input DMAs so later denominators take the ~13 us fast path.
            mk0_sb = res.tile([P, 1], F32, name="mk0_sb")
            nc.sync.dma_start(mk0_sb[:], mk_d[:])
            wu_in = dram.tile([P, 1], F32, name="wu_in")
            wu_out = dram.tile([P, 1], F32, name="wu_out")
            nc.gpsimd.dma_start(wu_in[:], mk0_sb[:])
            nc.gpsimd.collective_compute(
                "AllReduce", mybir.AluOpType.add,
                replica_groups=[list(range(NCORES))],
                ins=[wu_in.opt()], outs=[wu_out.opt()])

            # ---- resident loads -------------------------------------
            # Interleave the W-shard first halves with the hidden chunks
            # so row-tile 0's k-th matmul can fire as chunk k lands; the
            # second halves and the copy-path inputs stream afterwards.
            wc_sb = res.tile([P, kc], BF16, name="wc_sb")
            nc.sync.dma_start(wc_sb[:], wc_d[:])
            mk_sb = res.tile([P, 1], F32, name="mk_sb")
            nc.sync.dma_start(mk_sb[:], mk_d[:])
            ht = [res.tile([P, NROW], BF16, name=f"ht{k}") for k in range(kc)]
            wt = [res.tile([P, VS], BF16, name=f"wt{k}") for k in range(kc)]
            # hidden chunks first: the z matmuls pace on them and keep
            # the PE warm while the W shard streams in behind
            for k in range(kc):
                nc.sync.dma_start(ht[k][:], hT_d[k * P:(k + 1) * P, :])
            for k in range(kc):
                nc.sync.dma_start(wt[k][:, 0:HALF],
                                  wT_d[k * P:(k + 1) * P, 0:HALF])
            for k in range(kc):
                nc.sync.dma_start(wt[k][:, HALF:VS],
                                  wT_d[k * P:(k + 1) * P, HALF:VS])
            at_sb = res.tile([P, B * T], BF16, name="at_sb")
            nc.sync.dma_start(at_sb[:], at_d[:])
            sm_sb = res.tile([P, B * CV], BF16, name="sm_sb")
            nc.sync.dma_start(sm_sb[:], sm_d[:])

            q_pm = res.tile([P, MT], F32, name="q_pm")
            pc_tb = res.tile([P, B], F32, name="pc_tb")

            def z_block():
                # z = hidden @ W_copy.T in [p, m] layout, then
                # p_copy = 1/(1+exp(-z)), q = 1-p_copy.  p_copy leaves in
                # row-major order via a TensorE transpose (a scattered DMA
                # would cost thousands of 4-byte descriptors), then comes
                # back from DRAM in [t, b] layout — both DMAs contiguous.
                zp = psum.tile([P, HALF], F32, name="pt", tag="pt")
                for m in range(MT):
                    zcol = m * P + 124  # stays inside one PSUM bank
                    for k in range(kc):
                        nc.tensor.matmul(
                            zp[:, zcol:zcol + 1],
                            ht[k][:, m * P:(m + 1) * P],
                            wc_sb[:, k:k + 1],
                            start=(k == 0), stop=(k == kc - 1),
                        )
                zv = zp[:].rearrange("p (m c) -> p m c", c=P)[:, :, 124]
                u_pm = res.tile([P, MT], F32, name="u_pm")
                nc.scalar.activation(u_pm[:], zv,
                                     mybir.ActivationFunctionType.Exp,
                                     scale=-1.0)
                den_pm = res.tile([P, MT], F32, name="den_pm")
                nc.vector.tensor_scalar_add(den_pm[:], u_pm[:], 1.0)
                pc_pm = res.tile([P, MT], F32, name="pc_pm")
                nc.vector.reciprocal(pc_pm[:], den_pm[:])
                nc.vector.tensor_mul(q_pm[:], u_pm[:], pc_pm[:])

                ident = res.tile([P, P], F32, name="ident")
                make_identity(nc, ident[:])
                pc_ext = res.tile([P, P], F32, name="pc_ext")
                nc.vector.memset(pc_ext[:], 0.0)
                nc.vector.tensor_copy(pc_ext[:, 0:MT], pc_pm[:])
                s_ps = psum.tile([P, HALF], F32, name="pt", tag="pt")
                nc.tensor.transpose(s_ps[:, 0:P], pc_ext[:], ident[:])
                s_sb = res.tile([P, P], F32, name="s_sb")
                nc.vector.tensor_copy(s_sb[:], s_ps[:, 0:P])
                # s_sb[m, p] = p_copy[row m*128+p] -> row-major pc output
                nc.sync.dma_start(
                    pc_d.ap().rearrange("(m p) one -> m (p one)", m=MT),
                    s_sb[0:MT, :])
                nc.gpsimd.dma_start(
                    pc_tb[:],
                    pc_d.ap().rearrange("(t b) one -> t (b one)", t=P))

            def copy_path():
                # one matmul per batch; PSUM drained immediately so the
                # slot frees fast, scaled by p_copy[t, b] later
                cpt = psum.tile([P, HALF], F32, name="pt", tag="pt")
                for bb in range(B):
                    nc.tensor.matmul(
                        cpt[:, bb * P:bb * P + CV],
                        at_sb[:, bb * T:(bb + 1) * T],
                        sm_sb[:, bb * CV:(bb + 1) * CV],
                        start=True, stop=True)
                cp_raw = res.tile([P, B * P], F32, name="cp_raw")
                nc.vector.tensor_copy(cp_raw[:], cpt[:, 0:B * P])
                cp_sb = res.tile([P, B * CV], F32, name="cp_sb")
                for bb in range(B):
                    nc.vector.tensor_scalar(
                        cp_sb[:, bb * CV:(bb + 1) * CV],
                        cp_raw[:, bb * P:bb * P + CV],
                        pc_tb[:, bb:bb + 1], None, mybir.AluOpType.mult)
                nc.sync.dma_start(cp_d[:], cp_sb[:])

            # ---- phase B: vocab-shard softmax stream ----------------
            # Groups of row tiles share one denominator AllReduce; their
            # E tiles stay resident until the reduced sums return, then
            # are scaled in place and streamed out as bf16.  Groups
            # taper at the end so the final collective covers one tile
            # and the tail stays short.  Small latency-critical DMAs
            # ride the gpsimd/qAct queues, never behind the megabyte
            # streams on qSP.
            GROUPS = [4, 4, 4, 4]
            assert sum(GROUPS) == MT
            z_block()

            def scale_block(pend):
                # runs one group behind the compute stream: by now the
                # group's AllReduce has had a full group's compute time
                # to finish, so nothing below blocks an engine queue
                gi, gtiles, b_out, e_tiles = pend
                gn = len(gtiles)
                d_g = small.tile([P, G], F32, name="d_g")
                nc.gpsimd.dma_start(d_g[:, 0:gn], b_out[:])
                rec = small.tile([P, G], F32, name="rec")
                nc.vector.reciprocal(rec[:, 0:gn], d_g[:, 0:gn])
                f_g = small.tile([P, G], F32, name="f_g")
                nc.vector.tensor_mul(f_g[:, 0:gn],
                                     q_pm[:, gtiles[0]:gtiles[0] + gn],
                                     rec[:, 0:gn])
                for j, m in enumerate(gtiles):
                    e = e_tiles[m]
                    if j % 2 == 0:
                        # DVE scale, stream out on qSP
                        nc.vector.tensor_scalar(e[:], e[:], f_g[:, j:j + 1],
                                                None, mybir.AluOpType.mult)
                        nc.sync.dma_start(out_d[m * P:(m + 1) * P, :], e[:])
                    else:
                        # ACT scale, stream out on qAct — splits the
                        # serialized scale+store chain across engines
                        nc.scalar.mul(e[:], e[:], f_g[:, j:j + 1])
                        nc.scalar.dma_start(out_d[m * P:(m + 1) * P, :], e[:])

            pending = None
            g0 = 0
            for gi, gn in enumerate(GROUPS):
                gtiles = list(range(g0, g0 + gn))
                g0 += gn
                e_tiles = {}
                sg = small.tile([P, G], F32, name="sg")
                for j, m in enumerate(gtiles):
                    msl = slice(m * P, (m + 1) * P)
                    e = epool.tile([P, VS], BF16, name="e")
                    e_tiles[m] = e
                    s2 = small.tile([P, 2], F32, name="s2")
                    for h in range(2):
                        c0 = h * HALF
                        wdt = HALF if h == 0 else VS - HALF
                        pt = psum.tile([P, HALF], F32, name="pt", tag="pt")
                        for k in range(kc):
                            for off, gw in _groups(wdt):
                                nc.tensor.matmul(
                                    pt[:, off:off + gw],
                                    ht[k][:, msl],
                                    wt[k][:, c0 + off:c0 + off + gw],
                                    start=(k == 0), stop=(k == kc - 1),
                                )
                        if h == 0:
                            # PAD_IDX mask: per-core input, -1e9 on core 0
                            nc.vector.tensor_scalar(
                                pt[:, PAD_IDX:PAD_IDX + 1],
                                pt[:, PAD_IDX:PAD_IDX + 1],
                                mk_sb[:, 0:1], None, mybir.AluOpType.add)
                        nc.scalar.activation(
                            e[:, c0:c0 + wdt], pt[:, 0:wdt],
                            mybir.ActivationFunctionType.Exp,
                            accum_out=s2[:, h:h + 1])
                    nc.vector.tensor_add(sg[:, j:j + 1],
                                         s2[:, 0:1], s2[:, 1:2])
                    if m == 1:
                        # overlaps the attn/src_map DMA tail; PE is warm
                        copy_path()

                # contiguous [P, gn] bounce layout: AllReduce is
                # elementwise, so keep the SBUF-native partition-major
                # order and every DMA stays descriptor-friendly
                b_in = dram.tile([P, gn], F32, name=f"bin{gi}")
                b_out = dram.tile([P, gn], F32, name=f"bout{gi}")
                # qAct HW-DGE: fires right after the exps that made sg
                nc.scalar.dma_start(b_in[:], sg[:, 0:gn])
                nc.gpsimd.collective_compute(
                    "AllReduce", mybir.AluOpType.add,
                    replica_groups=[list(range(NCORES))],
                    ins=[b_in.opt()], outs=[b_out.opt()])
                if pending is not None:
                    scale_block(pending)
                pending = (gi, gtiles, b_out, e_tiles)
            scale_block(pending)


    nc.compile()
    return nc


def _build_rows(kc):
    """Row-sharded SPMD graph: each core owns 256 rows and the full
    (padded) vocab; W streams through SBUF in 2048-column quads; the
    softmax denominator is local, so there are no collectives at all."""
    nc = bacc.Bacc("TRN2", target_bir_lowering=False, debug=False,
                   num_devices=NCORES)

    htm_d = nc.dram_tensor("htm", [kc * P, RPC], BF16, kind="ExternalInput")
    wP_d = nc.dram_tensor("wP", [kc * P, VP], BF16, kind="ExternalInput")
    wc_d = nc.dram_tensor("wc", [P, kc], BF16, kind="ExternalInput")
    at_d = nc.dram_tensor("at", [P, B * B], BF16, kind="ExternalInput")
    sm_d = nc.dram_tensor("sm", [P, B * CV], BF16, kind="ExternalInput")

    out_d = nc.dram_tensor("out0", [RPC, V], BF16, kind="ExternalOutput")
    pc_d = nc.dram_tensor("pc", [RPC, 1], F32, kind="ExternalOutput")
    cp_d = nc.dram_tensor("cp", [B, B * CV], F32, kind="ExternalOutput")

    with tile.TileContext(nc) as tc:
        with (
            tc.tile_pool(name="res", bufs=1) as res,
            tc.tile_pool(name="small", bufs=3) as small,
            tc.tile_pool(name="wqpool", bufs=5) as wqpool,
            tc.tile_pool(name="psum", bufs=2, space="PSUM") as psum,
        ):
            # ---- tiny resident loads --------------------------------
            wc_sb = res.tile([P, kc], BF16, name="wc_sb")
            nc.sync.dma_start(wc_sb[:], wc_d[:])
            htm = [res.tile([P, RPC], BF16, name=f"htm{k}")
                   for k in range(kc)]
            for k in range(kc):
                nc.sync.dma_start(htm[k][:], htm_d[k * P:(k + 1) * P, :])
            at_sb = res.tile([P, B * B], BF16, name="at_sb")
            nc.sync.dma_start(at_sb[:], at_d[:])
            sm_sb = res.tile([P, B * CV], BF16, name="sm_sb")
            nc.sync.dma_start(sm_sb[:], sm_d[:])

            q_own = res.tile([P, MTC], F32, name="q_own")
            pc_tbo = res.tile([B, B], F32, name="pc_tbo")

            def z_block():
                # z for the core's own rows -> p_copy, q, and the
                # [t_local, b] layout via TensorE transpose + roundtrip
                zp = psum.tile([P, QW], F32, name="pt", tag="pt")
                for m in range(MTC):
                    zcol = m * P + 124
                    for k in range(kc):
                        nc.tensor.matmul(
                            zp[:, zcol:zcol + 1],
                            htm[k][:, m * P:(m + 1) * P],
                            wc_sb[:, k:k + 1],
                            start=(k == 0), stop=(k == kc - 1),
                        )
                zv = zp[:].rearrange("p (m c) -> p m c", c=P)[:, 0:MTC, 124]
                u_o = res.tile([P, MTC], F32, name="u_o")
                nc.scalar.activation(u_o[:], zv,
                                     mybir.ActivationFunctionType.Exp,
                                     scale=-1.0)
                den_o = res.tile([P, MTC], F32, name="den_o")
                nc.vector.tensor_scalar_add(den_o[:], u_o[:], 1.0)
                pc_o = res.tile([P, MTC], F32, name="pc_o")
                nc.vector.reciprocal(pc_o[:], den_o[:])
                nc.vector.tensor_mul(q_own[:], u_o[:], pc_o[:])

                ident = res.tile([P, P], F32, name="ident")
                make_identity(nc, ident[:])
                pc_ext = res.tile([P, P], F32, name="pc_ext")
                nc.vector.memset(pc_ext[:], 0.0)
                nc.vector.tensor_copy(pc_ext[:, 0:MTC], pc_o[:])
                s_ps = psum.tile([P, QW], F32, name="pt", tag="pt")
                nc.tensor.transpose(s_ps[:, 0:P], pc_ext[:], ident[:])
                s_sb = res.tile([P, P], F32, name="s_sb")
                nc.vector.tensor_copy(s_sb[:], s_ps[:, 0:P])
                nc.sync.dma_start(
                    pc_d.ap().rearrange("(m p) one -> m (p one)", m=MTC),
                    s_sb[0:MTC, :])
                nc.gpsimd.dma_start(
                    pc_tbo[:],
                    pc_d.ap().rearrange("(t b) one -> t (b one)", t=B))

            def copy_path():
                # copy_prob for the core's own 16 t-values; PSUM drained
                # immediately so the slot frees fast
                cpt = psum.tile([P, QW], F32, name="pt", tag="pt")
                for bb in range(B):
                    nc.tensor.matmul(
                        cpt[0:B, bb * P:bb * P + CV],
                        at_sb[:, bb * B:(bb + 1) * B],
                        sm_sb[:, bb * CV:(bb + 1) * CV],
                        start=True, stop=True)
                cp_raw = res.tile([B, B * P], BF16, name="cp_raw")
                nc.vector.tensor_copy(cp_raw[:], cpt[0:B, 0:B * P])
                cp_sb = res.tile([B, B * CV], BF16, name="cp_sb")
                for bb in range(B):
                    nc.vector.tensor_scalar(
                        cp_sb[:, bb * CV:(bb + 1) * CV],
                        cp_raw[:, bb * P:bb * P + CV],
                        pc_tbo[:, bb:bb + 1], None, mybir.AluOpType.mult)
                nc.gpsimd.dma_start(cp_d[:], cp_sb[:])  # casts bf16->f32

            z_block()

            # ---- main stream: 16 W quads, both row tiles per quad ---
            # W is read from HBM exactly once; each resident quad feeds
            # both row tiles.  E lives as two full-vocab bf16 tiles.
            e_big = [res.tile([P, V], BF16, name=f"ebig{m}")
                     for m in range(MTC)]
            s16a = res.tile([P, NQ], F32, name="s16a")
            s16b = res.tile([P, NQ], F32, name="s16b")
            s16 = [s16a, s16b]
            KPAIR = (kc + 1) // 2
            for q in range(NQ):
                wqc = []
                for j in range(KPAIR):
                    klo = 2 * j
                    kn = min(2, kc - klo)
                    t_ = wqpool.tile([P, 2, QW], BF16, name="wqc")
                    nc.sync.dma_start(
                        t_[:, 0:kn, :],
                        wP_d[klo * P:(klo + kn) * P,
                             q * QW:(q + 1) * QW].rearrange(
                                 "(kk p) n -> p kk n", p=P))
                    wqc.append(t_)
                for m in range(MTC):
                    msl = slice(m * P, (m + 1) * P)
                    pt = psum.tile([P, QW], F32, name="pt", tag="pt")
                    for k in range(kc):
                        for off, gw in _groups(QW):
                            nc.tensor.matmul(
                                pt[:, off:off + gw],
                                htm[k][:, msl],
                                wqc[k // 2][:, k % 2, off:off + gw],
                                start=(k == 0), stop=(k == kc - 1),
                            )
                    if q == 0:
                        # PAD_IDX column (static: every core holds col 1)
                        nc.vector.memset(pt[:, PAD_IDX:PAD_IDX + 1], NEG)
                    # last quad: exp only the 1280 real columns; the
                    # padded-W columns are never read
                    ew = QW if q < NQ - 1 else V - (NQ - 1) * QW
                    nc.scalar.activation(
                        e_big[m][:, q * QW:q * QW + ew], pt[:, 0:ew],
                        mybir.ActivationFunctionType.Exp,
                        accum_out=s16[m][:, q:q + 1])
                if q == 1:
                    copy_path()

            # ---- denominators + scale + store (tail) ----------------
            SCW = VP // 4
            for m in range(MTC):
                s_m = small.tile([P, 1], F32, name="s_m")
                nc.vector.tensor_reduce(s_m[:], s16[m][:],
                                        mybir.AxisListType.X,
                                        mybir.AluOpType.add)
                rec = small.tile([P, 1], F32, name="rec")
                nc.vector.reciprocal(rec[:], s_m[:])
                f_m = small.tile([P, 1], F32, name="f_m")
                nc.vector.tensor_mul(f_m[:], q_own[:, m:m + 1], rec[:])
                r0 = m * P
                for c4 in range(4):
                    lo = c4 * SCW
                    cw = min(SCW, V - lo)
                    if cw <= 0:
                        continue
                    ev = e_big[m][:, lo:lo + cw]
                    if (m * 4 + c4) % 2 == 0:
                        nc.vector.tensor_scalar(ev, ev, f_m[:], None,
                                                mybir.AluOpType.mult)
                        nc.sync.dma_start(out_d[r0:r0 + P, lo:lo + cw], ev)
                    else:
                        nc.scalar.mul(ev, ev, f_m[:])
                        nc.scalar.dma_start(out_d[r0:r0 + P, lo:lo + cw], ev)

    nc.compile()
    return nc


def _get_nc(kc):
    key = ("rows" if USE_ROWS else "vocab", kc)
    if key not in _CACHE:
        _CACHE[key] = _build_rows(kc) if USE_ROWS else _build(kc)
    return _CACHE[key]


def _prepare_in_maps(hidden, attn, src_map, W, b, W_copy, b_copy):
    hidden = np.asarray(hidden, dtype=np.float32)
    attn = np.asarray(attn, dtype=np.float32)
    src_map = np.asarray(src_map, dtype=np.float32)
    W = np.asarray(W, dtype=np.float32)
    b = np.asarray(b, dtype=np.float32)
    W_copy = np.asarray(W_copy, dtype=np.float32)
    b_copy = np.asarray(b_copy, dtype=np.float32)

    use_bias = bool(np.any(b != 0.0) or np.any(b_copy != 0.0))
    kc = D // P + (1 if use_bias else 0)

    # hidden.T (+ optional ones row for bias folding), bf16
    hT = np.zeros((kc * P, NROW), dtype=ml_dtypes.bfloat16)
    hT[:D, :] = hidden.T.astype(ml_dtypes.bfloat16)
    if use_bias:
        hT[D, :] = ml_dtypes.bfloat16(1.0)

    # W_copy.T chunks -> [P, kc]
    wc = np.zeros((P, kc), dtype=ml_dtypes.bfloat16)
    wc_full = np.zeros((kc * P,), dtype=np.float32)
    wc_full[:D] = W_copy[0, :]
    if use_bias:
        wc_full[D] = b_copy[0]
    wc[:, :] = wc_full.reshape(kc, P).T.astype(ml_dtypes.bfloat16)

    # attn[:, 300:].T in [a, (b t)] layout, padded to 128 rows
    at = np.zeros((P, B * T), dtype=ml_dtypes.bfloat16)
    # attn rows are r = t*B + b -> reshape (T, B, A) -> [a, b, t]
    a_tba = attn[:, CTX:].reshape(T, B, A)
    at[:A, :] = np.ascontiguousarray(
        a_tba.transpose(2, 1, 0)).reshape(A, B * T).astype(ml_dtypes.bfloat16)

    # src_map in [a, (b v)] layout, padded to 128 rows
    sm = np.zeros((P, B * CV), dtype=ml_dtypes.bfloat16)
    sm[:A, :] = np.ascontiguousarray(src_map).reshape(
        A, B * CV).astype(ml_dtypes.bfloat16)

    in_maps = []
    for c in range(NCORES):
        wT = np.zeros((kc * P, VS), dtype=ml_dtypes.bfloat16)
        wT[:D, :] = W[c * VS:(c + 1) * VS, :].T.astype(ml_dtypes.bfloat16)
        if use_bias:
            wT[D, :] = b[c * VS:(c + 1) * VS].astype(ml_dtypes.bfloat16)
        mk = np.zeros((P, 1), dtype=np.float32)
        if c == PAD_IDX // VS:
            mk[:, 0] = NEG if (0 <= PAD_IDX < VS) else 0.0
        in_maps.append({
            "hT": hT, "wT": wT, "wc": wc, "at": at, "sm": sm, "mk": mk,
        })
    return in_maps, kc


def _prepare_in_maps_rows(hidden, attn, src_map, W, b, W_copy, b_copy):
    hidden = np.asarray(hidden, dtype=np.float32)
    attn = np.asarray(attn, dtype=np.float32)
    src_map = np.asarray(src_map, dtype=np.float32)
    W = np.asarray(W, dtype=np.float32)
    b = np.asarray(b, dtype=np.float32)
    W_copy = np.asarray(W_copy, dtype=np.float32)
    b_copy = np.asarray(b_copy, dtype=np.float32)

    use_bias = bool(np.any(b != 0.0) or np.any(b_copy != 0.0))
    kc = D // P + (1 if use_bias else 0)

    hT = np.zeros((kc * P, NROW), dtype=ml_dtypes.bfloat16)
    hT[:D, :] = hidden.T.astype(ml_dtypes.bfloat16)
    if use_bias:
        hT[D, :] = ml_dtypes.bfloat16(1.0)

    wP = np.zeros((kc * P, VP), dtype=ml_dtypes.bfloat16)
    wP[:D, :V] = W.T.astype(ml_dtypes.bfloat16)
    if use_bias:
        wP[D, :V] = b.astype(ml_dtypes.bfloat16)

    wc = np.zeros((P, kc), dtype=ml_dtypes.bfloat16)
    wc_full = np.zeros((kc * P,), dtype=np.float32)
    wc_full[:D] = W_copy[0, :]
    if use_bias:
        wc_full[D] = b_copy[0]
    wc[:, :] = wc_full.reshape(kc, P).T.astype(ml_dtypes.bfloat16)

    sm = np.zeros((P, B * CV), dtype=ml_dtypes.bfloat16)
    sm[:A, :] = np.ascontiguousarray(src_map).reshape(
        A, B * CV).astype(ml_dtypes.bfloat16)

    a_tba = attn[:, CTX:].reshape(T, B, A)
    in_maps = []
    for c in range(NCORES):
        htm = np.ascontiguousarray(hT[:, c * RPC:(c + 1) * RPC])
        at = np.zeros((P, B * B), dtype=ml_dtypes.bfloat16)
        at[:A, :] = np.ascontiguousarray(
            a_tba[c * B:(c + 1) * B].transpose(2, 1, 0)).reshape(
                A, B * B).astype(ml_dtypes.bfloat16)
        in_maps.append({"htm": htm, "wP": wP, "wc": wc, "at": at, "sm": sm})
    return in_maps, kc


def _assemble(results):
    out = np.empty((NROW, V + CV), dtype=np.float32)
    if USE_ROWS:
        for c in range(NCORES):
            out[c * RPC:(c + 1) * RPC, :V] = \
                results[c]["out0"].astype(np.float32)
        cp = np.concatenate([results[c]["cp"] for c in range(NCORES)],
                            axis=0)
        out[:, V:] = cp.reshape(P, B, CV).reshape(NROW, CV)
        p_copy = np.concatenate([results[c]["pc"] for c in range(NCORES)],
                                axis=0)
    else:
        for c in range(NCORES):
            out[:, c * VS:(c + 1) * VS] = \
                results[c]["out0"].astype(np.float32)
        out[:, V:] = results[0]["cp"].reshape(P, B, CV).reshape(NROW, CV)
        p_copy = results[0]["pc"]
    return out, p_copy


def _run(inputs, trace=False, **kw):
    if USE_ROWS:
        in_maps, kc = _prepare_in_maps_rows(**inputs)
    else:
        in_maps, kc = _prepare_in_maps(**inputs)
    nc = _get_nc(kc)
    res = run_bass_kernel_spmd(nc, in_maps, core_ids=list(range(NCORES)),
                               trace=trace, **kw)
    return res


def kernel(hidden, attn, src_map, W, b, W_copy, b_copy):
    res = _run(dict(hidden=hidden, attn=attn, src_map=src_map, W=W, b=b,
                    W_copy=W_copy, b_copy=b_copy))
    return _assemble(res.results)
